# revision 34
# baseline (speedup 1.0000x reference)
"""Multi-head attention (B=2, S=2048, RES=1024, H=16) on 8 NeuronCores.

Sharding: batch*heads across cores. Core c handles batch c//4 and heads
4*(c%4) .. 4*(c%4)+3 (column-sharded QKV weights). No cross-core comm.

Per-core kernel (S=2048, K=1024, C=256 = 4 heads x 64). Fully pipelined
single-PSUM-pool structure: the ScalarE exp stream is the pacing engine
(~138us); everything else (projections, AV matmuls, tails) is interleaved
into its shadow on the PE/DVE/DMA engines.

  xT = transpose(x)                   PE transpose into PSUM (bitcast bf16
                                      views of the f32 proj psum ring)
  QT/KT per head [128, S] bf16        d-rows duplicated into 64:128 so the
                                      two 512-wide QK matmuls of a t-block
                                      run as concurrent PE row-tiles
  va[st] [128, 4*66] bf16             per head 64 V cols + ones col + pad
                                      (ones col makes the softmax sums ride
                                      the AV matmul into psum row 64)
  attention per (head, shi-half, m):  scoresT = K^T Q (bf16, row-tiled)
                                      at = exp(scores/8) bf16 on ScalarE
  AV: outT[66, 512] += va[tb]^T at    bf16, M=66; runs lagged one block
  behind QK/exp (software pipeline) so the first block's V-projection
  demand spreads out; projections stream through a deadline-ordered aux
  queue popped between attention ops. Weights arrive host-repacked as the
  SBUF image so each matrix is one DMA with 4KB descriptors.
  Tail per block: PSUM->bf16 copy, DMA xbar transpose to [s, d], rows
  scaled by 1/sums, output DMA (split across queues).
"""

import sys

if "/opt/trn_rl_repo" not in sys.path:
    sys.path.insert(0, "/opt/trn_rl_repo")

import numpy as np

B = 2
S = 2048
RES = 1024
HEADS = 16
HD = 64  # head dim
N_CORES = 8
HPC = 4  # heads per core
C = HPC * HD  # 256 per-core projected width
K = RES  # contraction dim of projections
NKT = K // 128  # 8 k-chunks
NST = S // 128  # 16 s-tiles / t-blocks
SH = 1024  # s-half size per attention block
NM = NST // 2  # 8 DoubleRow t-pair passes
VAUG = 66  # 64 V cols + ones col + zero pad
VPAD = 68  # per-head stride in va tiles (4*68=272 bytes, 16B-aligned j-stride)
LN2 = 0.6931471805599453

_CACHE: dict = {}


def _build_nc():
    import concourse.mybir as mybir
    import concourse.tile as tile
    from concourse import bacc
    from concourse.masks import make_identity

    f32 = mybir.dt.float32
    bf16 = mybir.dt.bfloat16
    fp8 = mybir.dt.float8e4
    AF = mybir.ActivationFunctionType
    DR = mybir.MatmulPerfMode.DoubleRow

    nc = bacc.Bacc(None)
    # x arrives host-transposed and (group, s-tile)-packed:
    # [p, g, st, kk, 128] = x[g*512 + st*128 + c, kk*128 + p] -- 2KB
    # descriptor lines per (p, g, st); 16 DMAs so the exact tiles the
    # first v_projs need land first
    x_in = nc.dram_tensor("x", [128, (S // 512) * K // 128 * 512], bf16,
                          kind="ExternalInput")
    # weights arrive host-repacked as the SBUF image [128, NKT*C]
    # (partition p, chunk kk, col c) <- W[kk*128+p, c]: one DMA per matrix
    # with 4KB descriptors instead of 8 tiles x 128 descriptors of 512B
    wq_in = nc.dram_tensor("wq", [128, NKT * C], bf16, kind="ExternalInput")
    wk_in = nc.dram_tensor("wk", [128, NKT * C], bf16, kind="ExternalInput")
    wv_in = nc.dram_tensor("wv", [128, NKT * C], bf16, kind="ExternalInput")
    # output also host-unpacked from the SBUF image [128, NST*C]:
    # one DMA per shi-half with 8KB descriptors instead of 16 tiles of
    # 128 x 1KB descriptors
    out_d = nc.dram_tensor("out", [128, NST * C], f32, kind="ExternalOutput")

    with tile.TileContext(nc) as tc:
        with (
            tc.tile_pool(name="persist", bufs=1) as persist,
            tc.tile_pool(name="work", bufs=1) as work,
            tc.tile_pool(name="ps", bufs=1, space="PSUM") as ps,
        ):
            # identity built directly in bf16 on GPSIMD (DVE boots ~2us
            # later than Pool; keep it off the warm-up critical path)
            ident = persist.tile([128, 128], bf16)
            make_identity(nc, ident)

            qt = [
                persist.tile([128, S], bf16, name=f"qt{h}", tag="qt", bufs=HPC)
                for h in range(HPC)
            ]
            kt = [
                persist.tile([128, S], bf16, name=f"kt{h}", tag="kt", bufs=HPC)
                for h in range(HPC)
            ]
            xT = persist.tile([128, NKT * S], bf16, name="xT")
            # [p, sgroup(4), stile(4), kk(8), c(128)]
            xT5 = xT.rearrange("p (g t k c) -> p g t k c", g=4, t=4, k=NKT)
            va = [
                persist.tile(
                    [128, HPC * VAUG], bf16, name=f"va{st}", tag="va", bufs=NST
                )
                for st in range(NST)
            ]
            va3 = [v.rearrange("p (h c) -> p h c", h=HPC) for v in va]
            out_all = persist.tile([128, NST * C], f32, name="out_all")
            # persistent oT ring: pad rows 64:80 (xbar tile padding) are
            # zeroed once (on GPSIMD, below) instead of per block on the DVE
            oT_ring = [
                persist.tile([80, SH], bf16, name=f"oT{i}") for i in range(4)
            ]

            # ---- PSUM budget (8 banks): sc 2x[128,1024]f32 (4) +
            # outp 2x[66,512]f32 (2) + pp 2x[128,512]f32 (2) ----

            # Warm the PE clock gate during the initial x DMA wait: HAM needs
            # ~3.4us of *sustained* matmul activity before it un-throttles
            # 1.2 -> 2.4 GHz, so burn ~4.5us of back-to-back N=128 matmuls
            # into an (otherwise unused this early) sc-tag psum tile.
            wm = ps.tile([128, SH], f32, name="warm", tag="sc", bufs=2)

            def warm_burst(n):
                for w in range(n):
                    nc.tensor.matmul(
                        wm[:, (w % 8) * 128 : (w % 8) * 128 + 128],
                        ident[:],
                        ident[:],
                        start=True,
                        stop=True,
                        skip_group_check=True,
                    )

            warm_burst(26)

            # Input DMAs spread across trigger rings, critical pieces first
            # (queue order = descriptor order): v_proj(0) needs wv + x(0,0);
            # later tiles stream in per-(g,st).
            w3 = {}
            wtile = {}
            for name in ("wv", "wk", "wq"):
                t_ = work.tile([128, NKT * C], bf16, name=name, tag=name, bufs=1)
                wtile[name] = t_
                w3[name] = t_.rearrange("p (k c) -> p k c", k=NKT)
            wv3, wk3, wq3 = w3["wv"], w3["wk"], w3["wq"]

            x_view = x_in.rearrange("p (g t k c) -> p g t k c", g=4, t=4, k=NKT)

            def x_load(g, t, eng):
                eng.dma_start(xT5[:, g, t, :, :], x_view[:, g, t, :, :])

            def w_half(name, dram, hf, eng):
                cs = slice(hf * 4 * C, (hf + 1) * 4 * C)
                eng.dma_start(wtile[name][:, cs], dram[:, cs])

            # Each weight split in two ring-parallel halves so the head of
            # every ring carries the first consumer's data: v_proj(0) needs
            # wv + x(0,0); first k-chunk needs wk (~12us later).
            w_half("wv", wv_in, 0, nc.scalar)   # scalar: wv_a wk_a wq_a
            w_half("wv", wv_in, 1, nc.sync)     # sync:   wv_b x00 x02 ...
            w_half("wk", wk_in, 1, nc.gpsimd)   # gpsimd: wk_b x01 x03 ...
            w_half("wk", wk_in, 0, nc.scalar)
            x_load(0, 0, nc.sync)
            x_load(0, 1, nc.gpsimd)
            x_load(0, 2, nc.sync)
            x_load(0, 3, nc.gpsimd)
            w_half("wq", wq_in, 0, nc.scalar)
            w_half("wq", wq_in, 1, nc.sync)
            for g in range(1, 4):
                for t in range(4):
                    x_load(g, t, nc.sync if t % 2 == 0 else nc.gpsimd)

            # init memsets AFTER the DMA triggers so they don't delay
            # descriptor generation. The first 4 va tiles (needed by the
            # preamble v_projs at ~13us) go on the DVE (idle until then);
            # the rest ride the GPSIMD behind its trigger queue.
            # va: zero everything (covers pad cols), then the ones col per
            # head; V-proj copies fill cols 0:HD later.
            for st in range(NST):
                eng = nc.vector if st < 4 else nc.gpsimd
                eng.memset(va[st][:], 0.0)
                eng.memset(va3[st][:, :, HD : HD + 1], 1.0)
            for t_ in oT_ring:
                nc.gpsimd.memset(t_[64:80, :], 0.0)

            v_emitted = [False] * NST

            def v_proj(st):
                v_emitted[st] = True
                vp = ps.tile([128, 512], f32, name=f"vp{st}", tag="pp", bufs=2)
                vps = vp[:, 0:C]
                g, tl = st // 4, st % 4
                for kk in range(NKT):
                    nc.tensor.matmul(
                        vps,
                        xT5[:, g, tl, kk, :],
                        wv3[:, kk, :],
                        start=(kk == 0),
                        stop=(kk == NKT - 1),
                    )
                nc.vector.tensor_copy(
                    va3[st][:, :, 0:HD], vps.rearrange("p (h c) -> p h c", h=HPC)
                )

            # Q/K projection chunks. Paired form sweeps kk with an sc-pair
            # inner loop so each weight chunk's LDWEIGHTS serves two N=512
            # matmuls (the repeat load of identical weights is cheap).
            proj_state = {}
            chunks_done = set()  # (which, half, sc) emitted-part-b

            def qk_proj_pair(which, w_t, dsts, half, scp, part):
                key = ("pair", which, half, scp)
                if part == 0:
                    pps = [
                        ps.tile(
                            [128, 512], f32, name=f"pjp_{which}{half}{scp}{s2}",
                            tag="pp", bufs=2,
                        )
                        for s2 in range(2)
                    ]
                    proj_state[key] = pps
                else:
                    pps = proj_state[key]
                for kk in range(part * 2, part * 2 + 2):
                    for s2 in range(2):
                        nc.tensor.matmul(
                            pps[s2][:],
                            w_t[:, kk, half * 128 : (half + 1) * 128],
                            xT5[:, 2 * scp + s2, :, kk, :],
                            start=(kk == 0),
                            stop=(kk == NKT - 1),
                        )
                if part == 3:
                    proj_state.pop(key)
                    stgs = []
                    # both psum-releasing casts first (they free the pp ring
                    # for the PE), dup copies after
                    for s2 in range(2):
                        stg = work.tile(
                            [128, 512], bf16, name=f"stgp_{which}{half}{scp}{s2}",
                            tag="stg", bufs=2,
                        )
                        nc.vector.tensor_copy(stg[:], pps[s2][:])
                        stgs.append(stg)
                    for s2 in range(2):
                        sc = 2 * scp + s2
                        cols = slice(sc * 512, (sc + 1) * 512)
                        for hh in range(2):
                            h = 2 * half + hh
                            nc.vector.tensor_copy(
                                dsts[h][0:HD, cols],
                                stgs[s2][hh * HD : (hh + 1) * HD, :],
                            )
                            nc.vector.tensor_copy(
                                dsts[h][HD:128, cols],
                                stgs[s2][hh * HD : (hh + 1) * HD, :],
                            )
                        chunks_done.add((which, half, sc))

            def qk_proj(which, w_t, dsts, half, sc, part):
                key = (which, half, sc)
                if part == 0:
                    pp = ps.tile(
                        [128, 512], f32, name=f"pj_{which}{half}{sc}", tag="pp",
                        bufs=2,
                    )
                    proj_state[key] = pp
                else:
                    pp = proj_state.pop(key)
                for kk in range(part * 4, part * 4 + 4):
                    nc.tensor.matmul(
                        pp[:],
                        w_t[:, kk, half * 128 : (half + 1) * 128],
                        xT5[:, sc, :, kk, :],
                        start=(kk == 0),
                        stop=(kk == NKT - 1),
                    )
                if part == 1:
                    stg = work.tile(
                        [128, 512], bf16, name=f"stg_{which}{half}{sc}",
                        tag="stg", bufs=2,
                    )
                    nc.vector.tensor_copy(stg[:], pp[:])
                    cols = slice(sc * 512, (sc + 1) * 512)
                    for hh in range(2):
                        h = 2 * half + hh
                        nc.vector.tensor_copy(
                            dsts[h][0:HD, cols], stg[hh * HD : (hh + 1) * HD, :]
                        )
                        nc.vector.tensor_copy(
                            dsts[h][HD:128, cols], stg[hh * HD : (hh + 1) * HD, :]
                        )
                    chunks_done.add(key)

            # ---- preamble ----
            for st in range(4):
                v_proj(st)
                # bridge the DMA/weight-paced preamble so the PE never sees
                # a HAM MID window of idle and re-throttles
                warm_burst(2)
            for part in range(2):
                qk_proj("k", wk3, kt, 0, 0, part)
                warm_burst(2)
            for sc in range(2):
                for part in range(2):
                    qk_proj("q", wq3, qt, 0, sc, part)
                warm_burst(2)
            chunks_done.add(("k", 0, 0))
            chunks_done.add(("q", 0, 0))
            chunks_done.add(("q", 0, 1))

            # ---- aux queue (deadline order) ----
            aux = []

            def add_v(st):
                aux.append(("v", st, 1400, lambda st=st: v_proj(st)))

            def add_chunk(which, w_t, dsts, half, sc):
                for part in range(2):
                    aux.append(
                        (
                            "c",
                            (which, half, sc),
                            1150,
                            lambda p=part: qk_proj(which, w_t, dsts, half, sc, p),
                        )
                    )

            def add_pair(which, w_t, dsts, half, scp):
                for part in range(4):
                    aux.append(
                        (
                            "c",
                            (which, half, scp),
                            1150,
                            lambda p=part: qk_proj_pair(
                                which, w_t, dsts, half, scp, p
                            ),
                        )
                    )

            add_v(4)
            add_v(5)
            add_chunk("k", wk3, kt, 0, 1)
            add_v(6)
            add_v(7)
            add_pair("k", wk3, kt, 0, 1)   # k chunks 2,3
            add_pair("q", wq3, qt, 0, 1)   # q chunks 2,3
            add_v(8)
            add_v(9)
            add_v(10)
            add_v(11)
            add_v(12)
            add_v(13)
            add_v(14)
            add_v(15)
            add_pair("k", wk3, kt, 1, 0)
            add_pair("k", wk3, kt, 1, 1)
            add_pair("q", wq3, qt, 1, 0)
            add_pair("q", wq3, qt, 1, 1)

            aux_state = [0.0, 0.0]  # popped-cost, budget

            def pop_aux_budget(budget_ns):
                aux_state[1] += budget_ns
                while aux and aux_state[0] + aux[0][2] * 0.5 < aux_state[1]:
                    item = aux.pop(0)
                    aux_state[0] += item[2]
                    item[3]()

            def pop_aux(n):
                for _ in range(n):
                    if aux:
                        item = aux.pop(0)
                        aux_state[0] += item[2]
                        item[3]()

            def pop_matching(pred):
                i = 0
                while i < len(aux):
                    if pred(aux[i]):
                        item = aux.pop(i)
                        aux_state[0] += item[2]
                        item[3]()
                    else:
                        i += 1

            def close_open_proj():
                # open psum proj groups always have their remaining parts at
                # the aux front (budget pops are FIFO); close them before any
                # out-of-order pop so pp-ring WAR order can't deadlock
                while proj_state and aux:
                    pop_aux(1)

            def need_chunk(which, half, sc):
                if (which, half, sc) in chunks_done:
                    return
                close_open_proj()
                pop_matching(
                    lambda it: it[0] == "c"
                    and it[1][0] == which
                    and it[1][1] == half
                    and (it[1][2] == sc or it[1][2] == sc // 2)
                )

            def need_v(m):
                if v_emitted[2 * m] and v_emitted[2 * m + 1]:
                    return
                close_open_proj()
                pop_matching(
                    lambda it: it[0] == "v" and it[1] in (2 * m, 2 * m + 1)
                )

            # ---- attention stream ----
            # (h3,s0) before (h2,s1) so the shi=0 output-DMA batch (2MB/2)
            # overlaps the last blocks instead of landing in the tail
            blocks = [(0, 0), (0, 1), (1, 0), (1, 1), (2, 0), (3, 0), (2, 1), (3, 1)]
            at_ring = {}
            outp_ring = {}
            tail_pending = []

            def emit_qk_exp(b, h, shi, m):
                s0 = shi * SH
                half = h // 2
                need_chunk("k", half, (2 * m) // 4)
                need_chunk("q", half, 2 * shi)
                need_chunk("q", half, 2 * shi + 1)
                at_t = work.tile(
                    [128, 2 * SH], bf16, name=f"at_{b}_{m}", tag="at", bufs=16
                )
                at3 = at_t.rearrange("p (j s) -> p j s", j=2)
                at_ring[(b, m)] = at3
                for jj in range(2):
                    tb = 2 * m + jj
                    scp = ps.tile(
                        [128, SH], f32, name=f"sc_{b}_{m}_{jj}", tag="sc", bufs=2
                    )
                    for scj in range(2):
                        dlo = scj * HD
                        nc.tensor.matmul(
                            scp[:, scj * 512 : (scj + 1) * 512],
                            kt[h][dlo : dlo + HD, tb * 128 : (tb + 1) * 128],
                            qt[h][dlo : dlo + HD, s0 + scj * 512 : s0 + (scj + 1) * 512],
                            start=True,
                            stop=True,
                            skip_group_check=True,
                        )
                    nc.scalar.activation(at3[:, jj, :], scp[:], AF.Exp, scale=0.125)

            def emit_av(bp, m):
                h, shi = blocks[bp]
                need_v(m)
                if m == 0:
                    # the last block runs at lag 4, overlapping the previous
                    # block's accumulation -- borrow the (idle by now) pp tag
                    # so the outp ring needn't hold 4 live accumulators
                    tag = "pp" if bp == len(blocks) - 1 else "outp"
                    for scj in range(2):
                        outp_ring[(bp, scj)] = ps.tile(
                            [VAUG, 512], f32, name=f"op_{bp}_{scj}", tag=tag,
                            bufs=2,
                        )
                at3 = at_ring.pop((bp, m))
                for jj in range(2):
                    tb = 2 * m + jj
                    for scj in range(2):
                        nc.tensor.matmul(
                            outp_ring[(bp, scj)][:],
                            va[tb][:, h * VAUG : h * VAUG + VAUG],
                            at3[:, jj, scj * 512 : (scj + 1) * 512],
                            start=(tb == 0),
                            stop=(tb == NST - 1),
                        )
                if m == NM - 1:
                    oT = oT_ring[bp % 4]
                    for scj in range(2):
                        nc.vector.tensor_copy(
                            oT[0:VAUG, scj * 512 : (scj + 1) * 512],
                            outp_ring.pop((bp, scj))[:],
                        )
                    trb = work.tile(
                        [128, (SH // 128) * 80], bf16, name=f"trb{bp}", tag="trb",
                        bufs=4,
                    )
                    trb3 = trb.rearrange("p (j c) -> p j c", j=SH // 128)
                    if bp == len(blocks) - 1:
                        # last block: split the transpose across both HWDGE
                        # rings (exp is done by now, scalar ring is free) so
                        # the tail chain shortens
                        nc.sync.dma_start_transpose(
                            trb3[:, 0:4, :], oT[0:80, 0:512]
                        )
                        nc.scalar.dma_start_transpose(
                            trb3[:, 4:8, :], oT[0:80, 512:SH]
                        )
                    else:
                        nc.sync.dma_start_transpose(trb3[:, :, :], oT[0:80, :])
                    tail_pending.append((bp, trb3))

            out_all3 = out_all.rearrange("p (sb c) -> p sb c", sb=NST)
            od3 = out_d.rearrange("p (sb c) -> p sb c", sb=NST)

            def emit_norm():
                bp, trb3 = tail_pending.pop(0)
                h, shi = blocks[bp]
                last_bp = bp == len(blocks) - 1
                cols = slice(h * HD, (h + 1) * HD)
                # fused per-half norm: one reciprocal [128,4] + one broadcast
                # multiply [128,4,64] per half (each half only depends on its
                # own transpose DMA)
                for hf in range(2):
                    jr = slice(hf * 4, hf * 4 + 4)
                    rs = work.tile(
                        [128, 4], f32, name=f"rs_{bp}_{hf}", tag="rs", bufs=4
                    )
                    nc.vector.reciprocal(rs[:], trb3[:, jr, HD])
                    sbs = slice(shi * 8 + hf * 4, shi * 8 + hf * 4 + 4)
                    nc.vector.tensor_tensor(
                        out_all3[:, sbs, cols],
                        trb3[:, jr, 0:HD],
                        rs[:, :, None].to_broadcast([128, 4, HD]),
                        mybir.AluOpType.mult,
                    )
                    if last_bp:
                        # tail: DMA each half as soon as its norm lands,
                        # split across both (now idle) HWDGE rings
                        eng = nc.sync if hf == 0 else nc.scalar
                        eng.dma_start(
                            od3[:, sbs, cols], out_all3[:, sbs, cols]
                        )
                if not last_bp:
                    # per-block output DMA: this block's 256KB column slice
                    # is final now; only the last block lands in the tail.
                    sbs = slice(shi * 8, (shi + 1) * 8)
                    nc.sync.dma_start(od3[:, sbs, cols], out_all3[:, sbs, cols])

            last = len(blocks) - 1
            for b, (h, shi) in enumerate(blocks):
                for m in range(NM):
                    emit_qk_exp(b, h, shi, m)
                    if b > 0 and m == NM - 1:
                        # at m==7 the AV tail emits the oT casts that release
                        # the outp psum ring for the NEXT block's AV -- put
                        # them in the DVE queue ahead of the aux-pop casts
                        emit_av(b - 1, m)
                        pop_aux(1)
                    else:
                        pop_aux(2 if b == 0 else 1)
                        if b > 0:
                            if m in (0, 4) and tail_pending:
                                emit_norm()
                            emit_av(b - 1, m)
                    if b == last and m >= 2:
                        # last block's AV runs at lag 2 to shorten the drain
                        emit_av(last, m - 2)

            # drain: last block's remaining AV, remaining tails
            pop_aux(len(aux))
            for m in range(NM - 2, NM):
                emit_av(last, m)
            while tail_pending:
                emit_norm()

    nc.finalize()
    return nc


def _get_nc():
    if "nc" not in _CACHE:
        _CACHE["nc"] = _build_nc()
    return _CACHE["nc"]


def kernel(x, Wq, Wk, Wv):
    import ml_dtypes
    from concourse import bass_utils

    bf = ml_dtypes.bfloat16
    x = np.asarray(x, dtype=np.float32).astype(bf)
    Wq = np.asarray(Wq, dtype=np.float32).astype(bf)
    Wk = np.asarray(Wk, dtype=np.float32).astype(bf)
    Wv = np.asarray(Wv, dtype=np.float32).astype(bf)

    nc = _get_nc()

    def repack(w, cols):
        # SBUF image [partition p, chunk kk, col c] <- W[kk*128+p, c]
        return np.ascontiguousarray(
            w[:, cols].reshape(NKT, 128, C).transpose(1, 0, 2).reshape(128, NKT * C)
        )

    def repack_x(xb):
        # [p, g, st, kk, c] <- x[g*512 + st*128 + c, kk*128 + p]
        xt = xb.T  # [K, S]
        return np.ascontiguousarray(
            xt.reshape(NKT, 128, 4, 4, 128)
            .transpose(1, 2, 3, 0, 4)
            .reshape(128, NKT * S)
        )

    in_maps = []
    for c in range(N_CORES):
        b = c // 4
        g = c % 4
        cols = slice(g * C, (g + 1) * C)
        in_maps.append(
            {
                "x": repack_x(x[b]),
                "wq": repack(Wq, cols),
                "wk": repack(Wk, cols),
                "wv": repack(Wv, cols),
            }
        )

    res = bass_utils.run_bass_kernel_spmd(nc, in_maps, list(range(N_CORES)))
    _CACHE["last_results"] = res

    out = np.empty((B, S, RES), dtype=np.float32)
    for c in range(N_CORES):
        b = c // 4
        g = c % 4
        o = res.results[c]["out"].reshape(128, NST, C).transpose(1, 0, 2)
        out[b, :, g * C : (g + 1) * C] = o.reshape(S, C)
    return out



# revision 35
# speedup vs baseline: 1.1852x; 1.1852x over previous
"""Multi-head attention (B=2, S=2048, RES=1024, H=16) on 8 NeuronCores.

Sharding: batch*heads across cores. Core c handles batch c//4 and heads
4*(c%4) .. 4*(c%4)+3 (column-sharded QKV weights). No cross-core comm.

Per-core kernel (S=2048, K=1024, C=256 = 4 heads x 64). Fully pipelined
single-PSUM-pool structure: the ScalarE exp stream is the pacing engine
(~138us); everything else (projections, AV matmuls, tails) is interleaved
into its shadow on the PE/DVE/DMA engines.

  xT = transpose(x)                   PE transpose into PSUM (bitcast bf16
                                      views of the f32 proj psum ring)
  QT/KT per head [128, S] bf16        d-rows duplicated into 64:128 so the
                                      two 512-wide QK matmuls of a t-block
                                      run as concurrent PE row-tiles
  va[st] [128, 4*66] bf16             per head 64 V cols + ones col + pad
                                      (ones col makes the softmax sums ride
                                      the AV matmul into psum row 64)
  attention per (head, shi-half, m):  scoresT = K^T Q (bf16, row-tiled)
                                      at = exp(scores/8) bf16 on ScalarE
  AV: outT[66, 512] += va[tb]^T at    bf16, M=66; runs lagged one block
  behind QK/exp (software pipeline) so the first block's V-projection
  demand spreads out; projections stream through a deadline-ordered aux
  queue popped between attention ops. Weights arrive host-repacked as the
  SBUF image so each matrix is one DMA with 4KB descriptors.
  Tail per block: PSUM->bf16 copy, DMA xbar transpose to [s, d], rows
  scaled by 1/sums, output DMA (split across queues).
"""

import sys

if "/opt/trn_rl_repo" not in sys.path:
    sys.path.insert(0, "/opt/trn_rl_repo")

import numpy as np

B = 2
S = 2048
RES = 1024
HEADS = 16
HD = 64  # head dim
N_CORES = 8
HPC = 4  # heads per core
C = HPC * HD  # 256 per-core projected width
K = RES  # contraction dim of projections
NKT = K // 128  # 8 k-chunks
NST = S // 128  # 16 s-tiles / t-blocks
SH = 1024  # s-half size per attention block
NM = NST // 2  # 8 DoubleRow t-pair passes
VAUG = 66  # 64 V cols + ones col + zero pad
VPAD = 68  # per-head stride in va tiles (4*68=272 bytes, 16B-aligned j-stride)
LN2 = 0.6931471805599453

_CACHE: dict = {}


def _build_nc():
    import concourse.mybir as mybir
    import concourse.tile as tile
    from concourse import bacc
    from concourse.masks import make_identity

    f32 = mybir.dt.float32
    bf16 = mybir.dt.bfloat16
    fp8 = mybir.dt.float8e4
    AF = mybir.ActivationFunctionType
    DR = mybir.MatmulPerfMode.DoubleRow

    nc = bacc.Bacc(None)
    # x arrives host-transposed and (group, s-tile)-packed:
    # [p, g, st, kk, 128] = x[g*512 + st*128 + c, kk*128 + p] -- 2KB
    # descriptor lines per (p, g, st); 16 DMAs so the exact tiles the
    # first v_projs need land first
    x_in = nc.dram_tensor("x", [128, (S // 512) * K // 128 * 512], bf16,
                          kind="ExternalInput")
    # weights arrive host-repacked as the SBUF image [128, NKT*C]
    # (partition p, chunk kk, col c) <- W[kk*128+p, c]: one DMA per matrix
    # with 4KB descriptors instead of 8 tiles x 128 descriptors of 512B
    wq_in = nc.dram_tensor("wq", [128, NKT * C], bf16, kind="ExternalInput")
    wk_in = nc.dram_tensor("wk", [128, NKT * C], bf16, kind="ExternalInput")
    wv_in = nc.dram_tensor("wv", [128, NKT * C], bf16, kind="ExternalInput")
    # output also host-unpacked from the SBUF image [128, NST*C]:
    # one DMA per shi-half with 8KB descriptors instead of 16 tiles of
    # 128 x 1KB descriptors
    out_d = nc.dram_tensor("out", [128, NST * C], f32, kind="ExternalOutput")

    with tile.TileContext(nc) as tc:
        with (
            tc.tile_pool(name="persist", bufs=1) as persist,
            tc.tile_pool(name="work", bufs=1) as work,
            tc.tile_pool(name="ps", bufs=1, space="PSUM") as ps,
        ):
            # identity built directly in bf16 on GPSIMD (DVE boots ~2us
            # later than Pool; keep it off the warm-up critical path)
            ident = persist.tile([128, 128], bf16)
            make_identity(nc, ident)

            qt = [
                persist.tile([128, S], bf16, name=f"qt{h}", tag="qt", bufs=HPC)
                for h in range(HPC)
            ]
            kt = [
                persist.tile([128, S], bf16, name=f"kt{h}", tag="kt", bufs=HPC)
                for h in range(HPC)
            ]
            xT = persist.tile([128, NKT * S], bf16, name="xT")
            # [p, sgroup(4), stile(4), kk(8), c(128)]
            xT5 = xT.rearrange("p (g t k c) -> p g t k c", g=4, t=4, k=NKT)
            va = [
                persist.tile(
                    [128, HPC * VAUG], bf16, name=f"va{st}", tag="va", bufs=NST
                )
                for st in range(NST)
            ]
            va3 = [v.rearrange("p (h c) -> p h c", h=HPC) for v in va]
            out_all = persist.tile([128, NST * C], f32, name="out_all")
            # persistent oT ring: pad rows 64:80 (xbar tile padding) are
            # zeroed once (on GPSIMD, below) instead of per block on the DVE
            oT_ring = [
                persist.tile([80, SH], bf16, name=f"oT{i}") for i in range(4)
            ]

            # ---- PSUM budget (8 banks): sc 2x[128,1024]f32 (4) +
            # outp 2x[66,512]f32 (2) + pp 2x[128,512]f32 (2) ----

            # Warm the PE clock gate during the initial x DMA wait: HAM needs
            # ~3.4us of *sustained* matmul activity before it un-throttles
            # 1.2 -> 2.4 GHz, so burn ~4.5us of back-to-back N=128 matmuls
            # into an (otherwise unused this early) sc-tag psum tile.
            wm = ps.tile([128, SH], f32, name="warm", tag="sc", bufs=2)

            def warm_burst(n):
                for w in range(n):
                    nc.tensor.matmul(
                        wm[:, (w % 8) * 128 : (w % 8) * 128 + 128],
                        ident[:],
                        ident[:],
                        start=True,
                        stop=True,
                        skip_group_check=True,
                    )

            warm_burst(16)

            # Input DMAs spread across trigger rings, critical pieces first
            # (queue order = descriptor order): v_proj(0) needs wv + x(0,0);
            # later tiles stream in per-(g,st).
            w3 = {}
            wtile = {}
            for name in ("wv", "wk", "wq"):
                t_ = work.tile([128, NKT * C], bf16, name=name, tag=name, bufs=1)
                wtile[name] = t_
                w3[name] = t_.rearrange("p (k c) -> p k c", k=NKT)
            wv3, wk3, wq3 = w3["wv"], w3["wk"], w3["wq"]

            x_view = x_in.rearrange("p (g t k c) -> p g t k c", g=4, t=4, k=NKT)

            def x_load(g, t, eng):
                eng.dma_start(xT5[:, g, t, :, :], x_view[:, g, t, :, :])

            def w_half(name, dram, hf, eng):
                cs = slice(hf * 4 * C, (hf + 1) * 4 * C)
                eng.dma_start(wtile[name][:, cs], dram[:, cs])

            # Each weight split in two ring-parallel halves so the head of
            # every ring carries the first consumer's data: v_proj(0) needs
            # wv + x(0,0); first k-chunk needs wk (~12us later).
            w_half("wv", wv_in, 0, nc.scalar)   # scalar: wv_a wk_a wq_a
            w_half("wv", wv_in, 1, nc.sync)     # sync:   wv_b x00 x02 ...
            w_half("wk", wk_in, 1, nc.gpsimd)   # gpsimd: wk_b x01 x03 ...
            w_half("wk", wk_in, 0, nc.scalar)
            x_load(0, 0, nc.sync)
            x_load(0, 1, nc.gpsimd)
            x_load(0, 2, nc.sync)
            x_load(0, 3, nc.gpsimd)
            w_half("wq", wq_in, 0, nc.scalar)
            w_half("wq", wq_in, 1, nc.sync)
            for g in range(1, 4):
                for t in range(4):
                    x_load(g, t, nc.sync if t % 2 == 0 else nc.gpsimd)

            # init memsets AFTER the DMA triggers so they don't delay
            # descriptor generation. The first 4 va tiles (needed by the
            # preamble v_projs at ~13us) go on the DVE (idle until then);
            # the rest ride the GPSIMD behind its trigger queue.
            # va: zero everything (covers pad cols), then the ones col per
            # head; V-proj copies fill cols 0:HD later.
            for st in range(NST):
                nc.gpsimd.memset(va[st][:], 0.0)
                nc.gpsimd.memset(va3[st][:, :, HD : HD + 1], 1.0)
            for t_ in oT_ring:
                nc.gpsimd.memset(t_[64:80, :], 0.0)

            v_emitted = [False] * NST

            def v_proj(st):
                v_emitted[st] = True
                vp = ps.tile([128, 512], f32, name=f"vp{st}", tag="pp", bufs=2)
                vps = vp[:, 0:C]
                g, tl = st // 4, st % 4
                for kk in range(NKT):
                    nc.tensor.matmul(
                        vps,
                        xT5[:, g, tl, kk, :],
                        wv3[:, kk, :],
                        start=(kk == 0),
                        stop=(kk == NKT - 1),
                    )
                nc.vector.tensor_copy(
                    va3[st][:, :, 0:HD], vps.rearrange("p (h c) -> p h c", h=HPC)
                )

            # Q/K projection chunks. Paired form sweeps kk with an sc-pair
            # inner loop so each weight chunk's LDWEIGHTS serves two N=512
            # matmuls (the repeat load of identical weights is cheap).
            proj_state = {}
            chunks_done = set()  # (which, half, sc) emitted-part-b

            def qk_proj_pair(which, w_t, dsts, half, scp, part):
                key = ("pair", which, half, scp)
                if part == 0:
                    pps = [
                        ps.tile(
                            [128, 512], f32, name=f"pjp_{which}{half}{scp}{s2}",
                            tag="pp", bufs=2,
                        )
                        for s2 in range(2)
                    ]
                    proj_state[key] = pps
                else:
                    pps = proj_state[key]
                for kk in range(part * 2, part * 2 + 2):
                    for s2 in range(2):
                        nc.tensor.matmul(
                            pps[s2][:],
                            w_t[:, kk, half * 128 : (half + 1) * 128],
                            xT5[:, 2 * scp + s2, :, kk, :],
                            start=(kk == 0),
                            stop=(kk == NKT - 1),
                        )
                if part == 3:
                    proj_state.pop(key)
                    stgs = []
                    # both psum-releasing casts first (they free the pp ring
                    # for the PE), dup copies after
                    for s2 in range(2):
                        stg = work.tile(
                            [128, 512], bf16, name=f"stgp_{which}{half}{scp}{s2}",
                            tag="stg", bufs=2,
                        )
                        nc.vector.tensor_copy(stg[:], pps[s2][:])
                        stgs.append(stg)
                    for s2 in range(2):
                        sc = 2 * scp + s2
                        cols = slice(sc * 512, (sc + 1) * 512)
                        for hh in range(2):
                            h = 2 * half + hh
                            nc.vector.tensor_copy(
                                dsts[h][0:HD, cols],
                                stgs[s2][hh * HD : (hh + 1) * HD, :],
                            )
                            nc.vector.tensor_copy(
                                dsts[h][HD:128, cols],
                                stgs[s2][hh * HD : (hh + 1) * HD, :],
                            )
                        chunks_done.add((which, half, sc))

            def qk_proj(which, w_t, dsts, half, sc, part):
                key = (which, half, sc)
                if part == 0:
                    pp = ps.tile(
                        [128, 512], f32, name=f"pj_{which}{half}{sc}", tag="pp",
                        bufs=2,
                    )
                    proj_state[key] = pp
                else:
                    pp = proj_state.pop(key)
                for kk in range(part * 4, part * 4 + 4):
                    nc.tensor.matmul(
                        pp[:],
                        w_t[:, kk, half * 128 : (half + 1) * 128],
                        xT5[:, sc, :, kk, :],
                        start=(kk == 0),
                        stop=(kk == NKT - 1),
                    )
                if part == 1:
                    stg = work.tile(
                        [128, 512], bf16, name=f"stg_{which}{half}{sc}",
                        tag="stg", bufs=2,
                    )
                    nc.vector.tensor_copy(stg[:], pp[:])
                    cols = slice(sc * 512, (sc + 1) * 512)
                    for hh in range(2):
                        h = 2 * half + hh
                        nc.vector.tensor_copy(
                            dsts[h][0:HD, cols], stg[hh * HD : (hh + 1) * HD, :]
                        )
                        nc.vector.tensor_copy(
                            dsts[h][HD:128, cols], stg[hh * HD : (hh + 1) * HD, :]
                        )
                    chunks_done.add(key)

            # ---- preamble ----
            for st in range(4):
                v_proj(st)
                # bridge the DMA/weight-paced preamble so the PE never sees
                # a HAM MID window of idle and re-throttles
                warm_burst(2)
            for part in range(2):
                qk_proj("k", wk3, kt, 0, 0, part)
                warm_burst(2)
            for sc in range(2):
                for part in range(2):
                    qk_proj("q", wq3, qt, 0, sc, part)
                warm_burst(2)
            chunks_done.add(("k", 0, 0))
            chunks_done.add(("q", 0, 0))
            chunks_done.add(("q", 0, 1))

            # ---- aux queue (deadline order) ----
            aux = []

            def add_v(st):
                aux.append(("v", st, 1400, lambda st=st: v_proj(st)))

            def add_chunk(which, w_t, dsts, half, sc):
                for part in range(2):
                    aux.append(
                        (
                            "c",
                            (which, half, sc),
                            1150,
                            lambda p=part: qk_proj(which, w_t, dsts, half, sc, p),
                        )
                    )

            def add_pair(which, w_t, dsts, half, scp):
                for part in range(4):
                    aux.append(
                        (
                            "c",
                            (which, half, scp),
                            1150,
                            lambda p=part: qk_proj_pair(
                                which, w_t, dsts, half, scp, p
                            ),
                        )
                    )

            add_v(4)
            add_v(5)
            add_chunk("k", wk3, kt, 0, 1)
            add_v(6)
            add_v(7)
            add_pair("k", wk3, kt, 0, 1)   # k chunks 2,3
            add_pair("q", wq3, qt, 0, 1)   # q chunks 2,3
            add_v(8)
            add_v(9)
            add_v(10)
            add_v(11)
            add_v(12)
            add_v(13)
            add_v(14)
            add_v(15)
            add_pair("k", wk3, kt, 1, 0)
            add_pair("k", wk3, kt, 1, 1)
            add_pair("q", wq3, qt, 1, 0)
            add_pair("q", wq3, qt, 1, 1)

            aux_state = [0.0, 0.0]  # popped-cost, budget

            def pop_aux_budget(budget_ns):
                aux_state[1] += budget_ns
                while aux and aux_state[0] + aux[0][2] * 0.5 < aux_state[1]:
                    item = aux.pop(0)
                    aux_state[0] += item[2]
                    item[3]()

            def pop_aux(n):
                for _ in range(n):
                    if aux:
                        item = aux.pop(0)
                        aux_state[0] += item[2]
                        item[3]()

            def pop_matching(pred):
                i = 0
                while i < len(aux):
                    if pred(aux[i]):
                        item = aux.pop(i)
                        aux_state[0] += item[2]
                        item[3]()
                    else:
                        i += 1

            def close_open_proj():
                # open psum proj groups always have their remaining parts at
                # the aux front (budget pops are FIFO); close them before any
                # out-of-order pop so pp-ring WAR order can't deadlock
                while proj_state and aux:
                    pop_aux(1)

            def need_chunk(which, half, sc):
                if (which, half, sc) in chunks_done:
                    return
                close_open_proj()
                pop_matching(
                    lambda it: it[0] == "c"
                    and it[1][0] == which
                    and it[1][1] == half
                    and (it[1][2] == sc or it[1][2] == sc // 2)
                )

            def need_v(m):
                if v_emitted[2 * m] and v_emitted[2 * m + 1]:
                    return
                close_open_proj()
                pop_matching(
                    lambda it: it[0] == "v" and it[1] in (2 * m, 2 * m + 1)
                )

            # ---- attention stream ----
            # (h3,s0) before (h2,s1) so the shi=0 output-DMA batch (2MB/2)
            # overlaps the last blocks instead of landing in the tail
            blocks = [(0, 0), (0, 1), (1, 0), (1, 1), (2, 0), (3, 0), (2, 1), (3, 1)]
            at_ring = {}
            outp_ring = {}
            tail_pending = []

            def emit_qk_exp(b, h, shi, m):
                s0 = shi * SH
                half = h // 2
                need_chunk("k", half, (2 * m) // 4)
                need_chunk("q", half, 2 * shi)
                need_chunk("q", half, 2 * shi + 1)
                at_t = work.tile(
                    [128, 2 * SH], bf16, name=f"at_{b}_{m}", tag="at", bufs=16
                )
                at3 = at_t.rearrange("p (j s) -> p j s", j=2)
                at_ring[(b, m)] = at3
                for jj in range(2):
                    tb = 2 * m + jj
                    scp = ps.tile(
                        [128, SH], f32, name=f"sc_{b}_{m}_{jj}", tag="sc", bufs=2
                    )
                    for scj in range(2):
                        dlo = scj * HD
                        nc.tensor.matmul(
                            scp[:, scj * 512 : (scj + 1) * 512],
                            kt[h][dlo : dlo + HD, tb * 128 : (tb + 1) * 128],
                            qt[h][dlo : dlo + HD, s0 + scj * 512 : s0 + (scj + 1) * 512],
                            start=True,
                            stop=True,
                            skip_group_check=True,
                        )
                    nc.scalar.activation(at3[:, jj, :], scp[:], AF.Exp, scale=0.125)

            def emit_av(bp, m):
                h, shi = blocks[bp]
                need_v(m)
                if m == 0:
                    # the last block runs at lag 4, overlapping the previous
                    # block's accumulation -- borrow the (idle by now) pp tag
                    # so the outp ring needn't hold 4 live accumulators
                    tag = "pp" if bp == len(blocks) - 1 else "outp"
                    for scj in range(2):
                        outp_ring[(bp, scj)] = ps.tile(
                            [VAUG, 512], f32, name=f"op_{bp}_{scj}", tag=tag,
                            bufs=2,
                        )
                at3 = at_ring.pop((bp, m))
                for jj in range(2):
                    tb = 2 * m + jj
                    for scj in range(2):
                        nc.tensor.matmul(
                            outp_ring[(bp, scj)][:],
                            va[tb][:, h * VAUG : h * VAUG + VAUG],
                            at3[:, jj, scj * 512 : (scj + 1) * 512],
                            start=(tb == 0),
                            stop=(tb == NST - 1),
                        )
                if m == NM - 1:
                    oT = oT_ring[bp % 4]
                    for scj in range(2):
                        nc.vector.tensor_copy(
                            oT[0:VAUG, scj * 512 : (scj + 1) * 512],
                            outp_ring.pop((bp, scj))[:],
                        )
                    trb = work.tile(
                        [128, (SH // 128) * 80], bf16, name=f"trb{bp}", tag="trb",
                        bufs=4,
                    )
                    trb3 = trb.rearrange("p (j c) -> p j c", j=SH // 128)
                    if bp == len(blocks) - 1:
                        # last block: split the transpose across both HWDGE
                        # rings (exp is done by now, scalar ring is free) so
                        # the tail chain shortens
                        nc.sync.dma_start_transpose(
                            trb3[:, 0:4, :], oT[0:80, 0:512]
                        )
                        nc.scalar.dma_start_transpose(
                            trb3[:, 4:8, :], oT[0:80, 512:SH]
                        )
                    else:
                        nc.sync.dma_start_transpose(trb3[:, :, :], oT[0:80, :])
                    tail_pending.append((bp, trb3))

            out_all3 = out_all.rearrange("p (sb c) -> p sb c", sb=NST)
            od3 = out_d.rearrange("p (sb c) -> p sb c", sb=NST)

            def emit_norm():
                bp, trb3 = tail_pending.pop(0)
                h, shi = blocks[bp]
                last_bp = bp == len(blocks) - 1
                cols = slice(h * HD, (h + 1) * HD)
                # fused per-half norm: one reciprocal [128,4] + one broadcast
                # multiply [128,4,64] per half (each half only depends on its
                # own transpose DMA)
                for hf in range(2):
                    jr = slice(hf * 4, hf * 4 + 4)
                    rs = work.tile(
                        [128, 4], f32, name=f"rs_{bp}_{hf}", tag="rs", bufs=4
                    )
                    nc.vector.reciprocal(rs[:], trb3[:, jr, HD])
                    sbs = slice(shi * 8 + hf * 4, shi * 8 + hf * 4 + 4)
                    nc.vector.tensor_tensor(
                        out_all3[:, sbs, cols],
                        trb3[:, jr, 0:HD],
                        rs[:, :, None].to_broadcast([128, 4, HD]),
                        mybir.AluOpType.mult,
                    )
                    if last_bp:
                        # tail: DMA each half as soon as its norm lands,
                        # split across both (now idle) HWDGE rings
                        eng = nc.sync if hf == 0 else nc.scalar
                        eng.dma_start(
                            od3[:, sbs, cols], out_all3[:, sbs, cols]
                        )
                if not last_bp:
                    # per-block output DMA: this block's 256KB column slice
                    # is final now; only the last block lands in the tail.
                    sbs = slice(shi * 8, (shi + 1) * 8)
                    nc.sync.dma_start(od3[:, sbs, cols], out_all3[:, sbs, cols])

            last = len(blocks) - 1
            for b, (h, shi) in enumerate(blocks):
                for m in range(NM):
                    emit_qk_exp(b, h, shi, m)
                    if b > 0 and m == NM - 1:
                        # at m==7 the AV tail emits the oT casts that release
                        # the outp psum ring for the NEXT block's AV -- put
                        # them in the DVE queue ahead of the aux-pop casts
                        emit_av(b - 1, m)
                        pop_aux(1)
                    else:
                        pop_aux(2 if b == 0 else 1)
                        if b > 0:
                            if m in (0, 4) and tail_pending:
                                emit_norm()
                            emit_av(b - 1, m)
                    if b == last and m >= 2:
                        # last block's AV runs at lag 2 to shorten the drain
                        emit_av(last, m - 2)

            # drain: last block's remaining AV, remaining tails
            pop_aux(len(aux))
            for m in range(NM - 2, NM):
                emit_av(last, m)
            while tail_pending:
                emit_norm()

    nc.finalize()
    return nc


def _get_nc():
    if "nc" not in _CACHE:
        _CACHE["nc"] = _build_nc()
    return _CACHE["nc"]


def kernel(x, Wq, Wk, Wv):
    import ml_dtypes
    from concourse import bass_utils

    bf = ml_dtypes.bfloat16
    x = np.asarray(x, dtype=np.float32).astype(bf)
    Wq = np.asarray(Wq, dtype=np.float32).astype(bf)
    Wk = np.asarray(Wk, dtype=np.float32).astype(bf)
    Wv = np.asarray(Wv, dtype=np.float32).astype(bf)

    nc = _get_nc()

    def repack(w, cols):
        # SBUF image [partition p, chunk kk, col c] <- W[kk*128+p, c]
        return np.ascontiguousarray(
            w[:, cols].reshape(NKT, 128, C).transpose(1, 0, 2).reshape(128, NKT * C)
        )

    def repack_x(xb):
        # [p, g, st, kk, c] <- x[g*512 + st*128 + c, kk*128 + p]
        xt = xb.T  # [K, S]
        return np.ascontiguousarray(
            xt.reshape(NKT, 128, 4, 4, 128)
            .transpose(1, 2, 3, 0, 4)
            .reshape(128, NKT * S)
        )

    in_maps = []
    for c in range(N_CORES):
        b = c // 4
        g = c % 4
        cols = slice(g * C, (g + 1) * C)
        in_maps.append(
            {
                "x": repack_x(x[b]),
                "wq": repack(Wq, cols),
                "wk": repack(Wk, cols),
                "wv": repack(Wv, cols),
            }
        )

    res = bass_utils.run_bass_kernel_spmd(nc, in_maps, list(range(N_CORES)))
    _CACHE["last_results"] = res

    out = np.empty((B, S, RES), dtype=np.float32)
    for c in range(N_CORES):
        b = c // 4
        g = c % 4
        o = res.results[c]["out"].reshape(128, NST, C).transpose(1, 0, 2)
        out[b, :, g * C : (g + 1) * C] = o.reshape(S, C)
    return out

